# revision 1
# baseline (speedup 1.0000x reference)
"""Trainium2 Bass kernel for nn_AttentionOut (causal MHA + output projection).

Problem: B=2, S=2048, D=1024, H=16 heads, Dh=64, fp32.
  out = softmax(causal(q@k^T/8)) @ v, projected by W_O, plus b_O.
  Returns (residual, out) like the reference.

Sharding (8 cores): batch x head-group tensor parallel. Core c handles
batch b=c//4 and heads [4g, 4g+4) with g=c%4. Each core computes its
4 heads' attention for all queries plus the partial output projection
(2048, 1024); the host sums the 4 partials per batch (the "all-reduce")
and adds b_O.

Per-core device algorithm (fp16 operands, fp32 PSUM accumulation;
end-to-end max rel err vs the fp32 reference ~4e-4):
  - scores are computed TRANSPOSED: sT[k, q] = sum_d kT[d,k] qT[d,q],
    with the two heads of a "pair" packed on the 128 partitions (d=64
    each) via row-tiled matmuls (tile_position (0,0)/(64,0)).
  - exp on ScalarE (scale=1/8 fused) straight into an fp16 SBUF tile;
    causal mask on diagonal 128x128 blocks via a 0/1 triangular mask
    multiply on VectorE (no max-subtraction needed: |scores| <= ~9).
  - AV: zT[dpair, q] accumulates in PSUM via col-tiled matmuls
    (tile_position (0,0)/(0,64)); the denominator sum_k exp accumulates
    in a separate PSUM bank via M=32 ones-matmuls at col positions 32h.
  - normalize: reciprocal(denoms) -> broadcast over d partitions via a
    selection matmul -> multiply into zTn (SBUF, fp16).
  - projection: out[q, m] = sum_pairs zTn^T @ W_O, interleaved into the
    next chunk's score loop to keep ScalarE fed.
"""

import sys
import numpy as np

sys.path.insert(0, "/opt/trn_rl_repo")

B, S, D = 2, 2048, 1024
NH, DH = 16, 64
P = 128
NCORES = 8
HPC = 4            # heads per core
NPAIR = 2          # head pairs per core
QCH = 512          # query chunk (columns of transposed scores)
NCHUNK = S // QCH  # 4
NJ = S // P        # 16 key blocks

_COMPILED = None


def _build():
    import concourse.bacc as bacc
    import concourse.mybir as mybir
    import concourse.tile as tile

    F32 = mybir.dt.float32
    F16 = mybir.dt.float16
    EXP = mybir.ActivationFunctionType.Exp

    nc = bacc.Bacc("TRN2", target_bir_lowering=False, debug=False,
                   num_devices=NCORES)

    qT_d = nc.dram_tensor("qT", [2 * P, S], F16, kind="ExternalInput")
    kT_d = nc.dram_tensor("kT", [2 * P, S], F16, kind="ExternalInput")
    v_d = nc.dram_tensor("v", [S, HPC, DH], F16, kind="ExternalInput")
    wo_d = nc.dram_tensor("wo", [2 * P, D], F16, kind="ExternalInput")
    tri_d = nc.dram_tensor("tri", [P, P], F16, kind="ExternalInput")
    sel_d = nc.dram_tensor("sel", [P, 2 * P], F16, kind="ExternalInput")
    out_d = nc.dram_tensor("outp", [S, D], F32, kind="ExternalOutput")

    with tile.TileContext(nc) as tc:
        with (
            tc.tile_pool(name="const", bufs=1) as cpool,
            tc.tile_pool(name="work", bufs=4) as wpool,
            tc.tile_pool(name="zn", bufs=8) as znpool,
            tc.tile_pool(name="ost", bufs=3) as opool,
            tc.tile_pool(name="psc", bufs=2, space="PSUM") as psc,
            tc.tile_pool(name="pz", bufs=2, space="PSUM") as pz,
            tc.tile_pool(name="pden", bufs=2, space="PSUM") as pden,
        ):
            kT_sb = cpool.tile([P, NPAIR, S], F16, tag="kT")
            qT_sb = cpool.tile([P, NPAIR, S], F16, tag="qT")
            v_sb = cpool.tile([P, NJ, HPC, DH], F16, tag="v")
            wo_sb = cpool.tile([P, NPAIR, D], F16, tag="wo")
            tri_sb = cpool.tile([P, P], F16, tag="tri")
            sel_sb = cpool.tile([P, 2 * P], F16, tag="sel")
            ones_sb = cpool.tile([P, 32], F16, tag="ones")

            # load order favors what the first chunk needs first
            nc.sync.dma_start(kT_sb[:, 0, 0:QCH], kT_d[0:P, 0:QCH])
            nc.sync.dma_start(qT_sb[:, 0, 0:QCH], qT_d[0:P, 0:QCH])
            v_re = v_d.rearrange("(j p) h e -> p j h e", p=P)
            nc.sync.dma_start(v_sb[:, 0:4], v_re[:, 0:4])
            nc.sync.dma_start(tri_sb[:], tri_d[:])
            nc.sync.dma_start(kT_sb[:, 0, QCH:S], kT_d[0:P, QCH:S])
            nc.sync.dma_start(qT_sb[:, 0, QCH:S], qT_d[0:P, QCH:S])
            nc.sync.dma_start(kT_sb[:, 1, :], kT_d[P:2 * P, :])
            nc.sync.dma_start(qT_sb[:, 1, :], qT_d[P:2 * P, :])
            nc.sync.dma_start(v_sb[:, 4:NJ], v_re[:, 4:NJ])
            nc.sync.dma_start(sel_sb[:], sel_d[:])
            nc.sync.dma_start(wo_sb[:], wo_d.rearrange("(c p) m -> p c m", p=P))
            nc.vector.memset(ones_sb[:], 1.0)

            # deferred projection work, interleaved into later chunks.
            # Two items stay reserved to cover the chunk-boundary
            # reciprocal latency with PE work.
            proj_queue = []

            def emit_proj_one(reserve=2):
                if len(proj_queue) > reserve:
                    proj_queue.pop(0)()

            def make_proj(c, zn_pair):
                def emit(qs, mc):
                    po = pden.tile([P, QCH], F32, tag="den", name="po")
                    for pair in range(NPAIR):
                        nc.tensor.matmul(
                            po[:],
                            zn_pair[pair][:, qs * P:(qs + 1) * P],
                            wo_sb[:, pair, mc * QCH:(mc + 1) * QCH],
                            start=(pair == 0), stop=(pair == 1),
                        )
                    ot = opool.tile([P, QCH], F32, tag="ot", name="ot")
                    nc.vector.tensor_copy(ot[:], po[:])
                    nc.sync.dma_start(
                        out_d[c * QCH + qs * P: c * QCH + (qs + 1) * P,
                              mc * QCH:(mc + 1) * QCH],
                        ot[:])
                return [lambda qs=qs, mc=mc: emit(qs, mc)
                        for qs in range(QCH // P) for mc in range(D // QCH)]

            # chunk-boundary normalize, deferred into the next chunk's
            # pipeline so PE keeps streaming while the DVE reciprocal runs
            pending_norm = [None]

            def make_norm(c, z_pair, den):
                def run():
                    rec = wpool.tile([P, QCH], F16, tag="rec", name="rec")
                    with nc.allow_low_precision(
                            reason="fp16 softmax reciprocal (~5e-4) in "
                                   "budget"):
                        nc.vector.reciprocal(rec[:], den[:])
                    zn_pair = []
                    for pair in range(NPAIR):
                        bc = pden.tile([P, QCH], F32, tag="den", name="bc")
                        nc.tensor.matmul(
                            bc[:],
                            sel_sb[:, pair * P:(pair + 1) * P],
                            rec[:],
                            start=True, stop=True)
                        bcs = wpool.tile([P, QCH], F32, tag="bcs",
                                         name="bcs")
                        nc.vector.tensor_copy(bcs[:], bc[:])
                        zn = znpool.tile([P, QCH], F16, tag="zn", name="zn")
                        with nc.allow_low_precision(
                                reason="fp16 z normalize (~5e-4) in budget"):
                            nc.vector.tensor_tensor(
                                zn[:], z_pair[pair][:], bcs[:],
                                mybir.AluOpType.mult)
                        zn_pair.append(zn)
                    proj_queue.extend(make_proj(c, zn_pair))
                return run

            for c in range(NCHUNK):
                jmax = 4 * (c + 1)
                z_pair = None
                den = None

                for pair in range(NPAIR):
                    # software-pipelined emission: AV/den for iteration j
                    # are emitted during iteration j+1, so the in-order PE
                    # queue never stalls waiting for exp(j) on ScalarE.
                    pending = None

                    def emit_avden(pj, pqoff, pqlen, pexpT):
                        for par in range(2):
                            h = 2 * pair + par
                            nc.tensor.matmul(
                                z_pair[pair][64 * par:64 * par + 64,
                                             pqoff:QCH],
                                v_sb[:, pj, h, :],
                                pexpT[:, par, :pqlen],
                                start=(pj == 0),
                                stop=(pj == jmax - 1),
                                tile_position=(0, 64 * par),
                                skip_group_check=True,
                            )
                        for par in range(2):
                            h = 2 * pair + par
                            nc.tensor.matmul(
                                den[32 * h:32 * h + 32, pqoff:QCH],
                                ones_sb[:, 0:32],
                                pexpT[:, par, :pqlen],
                                start=(pj == 0),
                                stop=(pj == jmax - 1),
                                tile_position=(0, 32 * h),
                                skip_group_check=True,
                            )

                    for j in range(jmax):
                        qoff = max(0, P * j - QCH * c)
                        qlen = QCH - qoff
                        diag = P * j >= QCH * c
                        q0 = QCH * c + qoff

                        sc = psc.tile([P, 2, QCH], F32, tag="sc", name="sc")
                        for par in range(2):
                            nc.tensor.matmul(
                                sc[:, par, :qlen],
                                kT_sb[64 * par:64 * par + 64, pair,
                                      P * j:P * (j + 1)],
                                qT_sb[64 * par:64 * par + 64, pair,
                                      q0:q0 + qlen],
                                start=True, stop=True,
                                tile_position=(64 * par, 0),
                            )
                        expT = wpool.tile([P, 2, QCH], F16, tag="expT",
                                          name="expT")
                        nc.scalar.activation(
                            expT[:, :, :qlen], sc[:, :, :qlen], EXP,
                            scale=0.125)
                        if diag:
                            nc.vector.tensor_tensor(
                                expT[:, :, 0:P], expT[:, :, 0:P],
                                tri_sb[:, None, :].to_broadcast((P, 2, P)),
                                mybir.AluOpType.mult)
                        if pair == 0 and j == 0:
                            # previous chunk's normalize runs here, hidden
                            # behind this chunk's first scores + reserved
                            # projection matmuls
                            if pending_norm[0] is not None:
                                emit_proj_one(reserve=0)
                                emit_proj_one(reserve=0)
                                pending_norm[0]()
                                pending_norm[0] = None
                            z_pair = [pz.tile([P, QCH], F32, tag="z",
                                              name=f"z_c{c}p{i}")
                                      for i in range(NPAIR)]
                            den = pden.tile([P, QCH], F32, tag="den",
                                            name="den")
                        if pending is not None:
                            emit_avden(*pending)
                            emit_proj_one()
                        pending = (j, qoff, qlen, expT)
                    emit_avden(*pending)
                    emit_proj_one()

                pending_norm[0] = make_norm(c, z_pair, den)

            pending_norm[0]()
            while proj_queue:
                proj_queue.pop(0)()

    nc.compile()
    return nc


def _prep_inputs(c, q, k, v, W_O):
    b, g = c // 4, c % 4
    hs = slice(g * HPC * DH, (g + 1) * HPC * DH)
    qT = np.ascontiguousarray(q[b][:, hs].T.astype(np.float16))
    kT = np.ascontiguousarray(k[b][:, hs].T.astype(np.float16))
    vh = np.ascontiguousarray(
        v[b][:, hs].reshape(S, HPC, DH).astype(np.float16))
    wo = np.ascontiguousarray(
        W_O[g * HPC:(g + 1) * HPC].reshape(HPC * DH, D).astype(np.float16))
    tri = np.triu(np.ones((P, P), dtype=np.float16))
    sel = np.zeros((P, 2 * P), dtype=np.float16)
    for pair in range(NPAIR):
        sel[64 * pair, pair * P: pair * P + 64] = 1.0
        sel[64 * pair + 32, pair * P + 64: (pair + 1) * P] = 1.0
    return {"qT": qT, "kT": kT, "v": vh, "wo": wo, "tri": tri, "sel": sel}


def _get_compiled():
    global _COMPILED
    if _COMPILED is None:
        _COMPILED = _build()
    return _COMPILED


def kernel(residual, q, k, v, W_O, b_O, _trace=False, _trace_cores=None):
    from concourse.bass_utils import run_bass_kernel_spmd

    residual = np.asarray(residual, dtype=np.float32)
    q = np.asarray(q, dtype=np.float32)
    k = np.asarray(k, dtype=np.float32)
    v = np.asarray(v, dtype=np.float32)
    W_O = np.asarray(W_O, dtype=np.float32)
    b_O = np.asarray(b_O, dtype=np.float32)

    nc = _get_compiled()
    core_ids = list(range(NCORES))
    in_maps = [_prep_inputs(c, q, k, v, W_O) for c in core_ids]
    kw = {}
    if _trace:
        kw = dict(trace=True,
                  trace_cores=_trace_cores or core_ids)
    res = run_bass_kernel_spmd(nc, in_maps, core_ids, **kw)

    out = np.zeros((B, S, D), dtype=np.float32)
    for c in core_ids:
        out[c // 4] += res.results[c]["outp"]
    out += b_O
    if _trace:
        kernel.last_result = res
    return (residual, out)

